# revision 5
# baseline (speedup 1.0000x reference)
"""Trainium2 Bass kernel for nn_MessageLayer (GNN message passing).

Reference computation (per edge, E=1.6M, H=16, DE=32):
    A = (e @ W1 + b1).reshape(E, 16, 16)
    m[e,i] = sum_j A[e,i,j] * h[e,j]  +  (e @ W2 + b2)[e,i]

Strategy ("layout B", pure data-parallel over E across 8 cores):
  * Host preps transposed fp16 inputs: eT [32,Ec], hT1 [17,Ec] (h.T + ones row).
  * Per 500-edge chunk, partitions = the 256 columns of A^T (two halves of
    128, p=(i,j) with i major), free dim = edges:
      - PE: psumA/psumB [128,F] = W1half.T @ eT     (A^T without b1)
      - PE: psumH [128,F] = Rep.T @ hT              (h replicated over i)
      - ACT: sbufH = copy(psumH)                    (PSUM x PSUM TT illegal)
      - DVE: tmpA/tmpB [128,F] fp16 = psumA/B * sbufH   (the irreducible
        elementwise multiply of the einsum)
      - PE into psumM [16,F] (PSUM accumulation):
          Wcomb.T @ [eT; hT; ones]   (e@W2 + b2 + sum_j b1[i,j] h_j)
        + Ga.T @ tmpA + Gb.T @ tmpB  (group-sum over j = the einsum reduce)
      - ACT: mout = copy(psumM); DMA mout -> mT [16,Ec] HBM
  * Host transposes mT back.

All matmuls use fp16 operands (fp32 matmul is 4x slower on the PE; fp16
keeps input rounding at 2^-11). PSUM accumulation is fp32.
"""

import numpy as np

import concourse.bass as bass
import concourse.mybir as mybir
import concourse.tile as tile
from concourse import bacc
from concourse.bass_utils import run_bass_kernel_spmd

H = 16
DE = 32
NCORES = 8
F = 500  # edges per chunk (matmul free dim; 500*4B = 2000B <= one PSUM bank)

KX = DE + H + 1  # stacked input rows: eT(32) + hT(16) + ones(1)

f16 = mybir.dt.float16
f32 = mybir.dt.float32


def build_program(n_chunks: int):
    """Build the SPMD Bass program for one core processing n_chunks*F edges."""
    Ec = n_chunks * F
    nc = bacc.Bacc("TRN2", target_bir_lowering=False, debug=False)

    eT_d = nc.dram_tensor("eT", [DE, Ec], f16, kind="ExternalInput")
    hT1_d = nc.dram_tensor("hT1", [H + 1, Ec], f16, kind="ExternalInput")
    wa_d = nc.dram_tensor("wa", [DE, 128], f16, kind="ExternalInput")
    wb_d = nc.dram_tensor("wb", [DE, 128], f16, kind="ExternalInput")
    rep_d = nc.dram_tensor("rep", [H, 128], f16, kind="ExternalInput")
    wcomb_d = nc.dram_tensor("wcomb", [KX, H], f16, kind="ExternalInput")
    ga_d = nc.dram_tensor("ga", [128, H], f16, kind="ExternalInput")
    gb_d = nc.dram_tensor("gb", [128, H], f16, kind="ExternalInput")
    mT_d = nc.dram_tensor("mT", [H, Ec], f32, kind="ExternalOutput")

    mul = mybir.AluOpType.mult

    with tile.TileContext(nc) as tc:
        with (
            tc.tile_pool(name="const", bufs=1) as cpool,
            tc.tile_pool(name="io", bufs=4) as iopool,
            tc.tile_pool(name="work", bufs=3) as wpool,
            tc.tile_pool(name="psa", bufs=2, space="PSUM") as psa,
            tc.tile_pool(name="psb", bufs=2, space="PSUM") as psb,
            tc.tile_pool(name="psh", bufs=2, space="PSUM") as psh,
            tc.tile_pool(name="psm", bufs=2, space="PSUM") as psm,
        ):
            wa_s = cpool.tile([DE, 128], f16, tag="wa")
            wb_s = cpool.tile([DE, 128], f16, tag="wb")
            rep_s = cpool.tile([H, 128], f16, tag="rep")
            wcomb_s = cpool.tile([KX, H], f16, tag="wcomb")
            ga_s = cpool.tile([128, H], f16, tag="ga")
            gb_s = cpool.tile([128, H], f16, tag="gb")
            nc.scalar.dma_start(wa_s[:], wa_d[:])
            nc.scalar.dma_start(wb_s[:], wb_d[:])
            nc.scalar.dma_start(rep_s[:], rep_d[:])
            nc.scalar.dma_start(wcomb_s[:], wcomb_d[:])
            nc.scalar.dma_start(ga_s[:], ga_d[:])
            nc.scalar.dma_start(gb_s[:], gb_d[:])

            for c in range(n_chunks):
                sl = slice(c * F, (c + 1) * F)

                x = iopool.tile([KX, F], f16, tag="x")
                nc.scalar.dma_start(x[0:DE, :], eT_d[:, sl])
                nc.scalar.dma_start(x[DE:KX, :], hT1_d[:, sl])
                # separate copy of hT at partition base 0 for the Rep matmul
                h0 = iopool.tile([H, F], f16, tag="h0")
                nc.scalar.dma_start(h0[:], hT1_d[0:H, sl])

                pa = psa.tile([128, F], f32, tag="pa")
                pb = psb.tile([128, F], f32, tag="pb")
                ph = psh.tile([128, F], f32, tag="ph")
                pm = psm.tile([H, F], f32, tag="pm")

                nc.tensor.matmul(pa[:], wa_s[:], x[0:DE, :], start=True, stop=True)
                nc.tensor.matmul(pb[:], wb_s[:], x[0:DE, :], start=True, stop=True)
                nc.tensor.matmul(ph[:], rep_s[:], h0[:], start=True, stop=True)

                sh = wpool.tile([128, F], f32, tag="sh")
                nc.scalar.copy(sh[:], ph[:])

                ta = wpool.tile([128, F], f16, tag="ta")
                tb = wpool.tile([128, F], f16, tag="tb")
                nc.vector.tensor_tensor(ta[:], pa[:], sh[:], mul)
                nc.vector.tensor_tensor(tb[:], pb[:], sh[:], mul)

                nc.tensor.matmul(pm[:], wcomb_s[:], x[:, :], start=True, stop=False)
                nc.tensor.matmul(pm[:], ga_s[:], ta[:], start=False, stop=False)
                nc.tensor.matmul(pm[:], gb_s[:], tb[:], start=False, stop=True)

                mo = wpool.tile([H, F], f32, tag="mo")
                nc.scalar.copy(mo[:], pm[:])
                nc.scalar.dma_start(mT_d[:, sl], mo[:])

    nc.compile()
    return nc


def host_prep_weights(W1, b1, W2, b2):
    """Rearrange the dense weights for the device program (fp16)."""
    W1 = np.asarray(W1, np.float32)
    b1 = np.asarray(b1, np.float32)
    W2 = np.asarray(W2, np.float32)
    b2 = np.asarray(b2, np.float32)

    wa = W1[:, :128].astype(np.float16)  # A^T half a: columns (i,j), i<8
    wb = W1[:, 128:].astype(np.float16)  # half b: i>=8

    # Rep[j, p] = 1 where p%16 == j  -> psumH[p=(i,j), e] = h[e, j]
    rep = np.zeros((H, 128), np.float16)
    for p in range(128):
        rep[p % H, p] = 1.0

    # Wcomb rows: [W2 (32); B1r (16); b2 (1)] against X rows [eT; hT; ones]
    # B1r[j, i] = b1[i*16+j]  (the b1 part of A times h)
    b1r = b1.reshape(H, H).T  # b1[(i,j)] -> [j, i]
    wcomb = np.concatenate(
        [W2, b1r, b2.reshape(1, H)], axis=0
    ).astype(np.float16)

    # Group indicators: Ga[p, i] = 1 iff p//16 == i ; Gb shifted by 8
    ga = np.zeros((128, H), np.float16)
    gb = np.zeros((128, H), np.float16)
    for p in range(128):
        ga[p, p // H] = 1.0
        gb[p, p // H + 8] = 1.0

    return dict(wa=wa, wb=wb, rep=rep, wcomb=wcomb, ga=ga, gb=gb)


def host_prep_inputs(h, e, Ec_pad):
    """Full [E,*] inputs -> per-core transposed fp16 arrays, padded to Ec_pad."""
    E = e.shape[0]
    per = E // NCORES
    ins = []
    for c in range(NCORES):
        sl = slice(c * per, (c + 1) * per)
        ec = np.zeros((DE, Ec_pad), np.float16)
        ec[:, :per] = e[sl].T.astype(np.float16)
        hc = np.zeros((H + 1, Ec_pad), np.float16)
        hc[:H, :per] = h[sl].T.astype(np.float16)
        hc[H, :] = 1.0
        ins.append((ec, hc))
    return ins


_CACHE = {}


def _get_program(n_chunks):
    if n_chunks not in _CACHE:
        _CACHE[n_chunks] = build_program(n_chunks)
    return _CACHE[n_chunks]


def kernel(h, e, W1, b1, W2, b2, _trace=False):
    h = np.asarray(h, np.float32)
    e = np.asarray(e, np.float32)
    E = e.shape[0]
    assert E % NCORES == 0
    per = E // NCORES
    n_chunks = (per + F - 1) // F
    Ec_pad = n_chunks * F

    nc = _get_program(n_chunks)
    w = host_prep_weights(W1, b1, W2, b2)
    ins = host_prep_inputs(h, e, Ec_pad)

    in_maps = []
    for c in range(NCORES):
        ec, hc = ins[c]
        in_maps.append(dict(eT=ec, hT1=hc, **w))

    res = run_bass_kernel_spmd(
        nc, in_maps, core_ids=list(range(NCORES)), trace=_trace
    )

    out = np.empty((E, H), np.float32)
    for c in range(NCORES):
        mT = res.results[c]["mT"]  # [H, Ec_pad] fp32
        out[c * per : (c + 1) * per] = mT[:, :per].T
    if _trace:
        return out, res
    return out
